# revision 21
# baseline (speedup 1.0000x reference)
"""Trainium2 Bass kernel for NonLinearSelfAttention.

Computes, per batch b:
    S    = x_b @ x_b.T * C**-0.5          [N, N]
    P    = softmax(S, axis=-1)
    out  = (P @ x_b) @ W.T + bias         [N, OUT]

Sharding: batch-data-parallel, one batch per NeuronCore (8 cores).

Per-core algorithm (N=4096, C=128):
  - E = exp(scale*S) is symmetric, so the tile E[J-block, A-block] computed in
    [j, i] layout is directly the lhsT needed by the P@V matmul for output
    block A — no transposes in the main loop.
  - The Linear folds through the attention: y = (E @ [z | 1]) / r + bias with
    z = x @ W.T, because (P x) W.T = P (x W.T).  The appended ones column
    produces the softmax row-sums r in per-partition layout for free
    (r_i = sum_j E[j, i] = sum_j E[i, j] by symmetry).
  - No max-subtraction needed: logits are ~N(0,1) with diagonal ~ sqrt(C)+,
    max ~ 20, exp(20) is well inside fp32 range.
  - exp is split between ScalarE (native, exact — always covers the diagonal
    blocks, which dominate softmax) and VectorE (Schraudolph bit-trick:
    bf16 bits of e^u ~= round(u*K1 + K2) as int16, bitcast to bf16; ~5%
    worst-case pointwise on off-diagonal mass, ~5e-3 end-to-end).  A running
    balance counter splits each slab's free range between the engines.
  - x is truncated (not rounded) to bf16 by copying the high u16 half of each
    fp32; the systematic -2^-9 relative bias is compensated in the exp scale
    (squared, since both matmul operands shrink) and in W.
"""
import numpy as np

import concourse.bass as bass
import concourse.tile as tile
from concourse.masks import make_identity
from concourse import bacc, mybir
from concourse import bass_utils

B = 8          # batches = cores
N = 4096       # sequence length
C = 128        # feature dim
OUT = 128      # linear out dim
NT = N // 128  # 32 j-tiles
QW = 512       # i-columns processed per quad-block
NQ = N // QW   # 8 quad blocks
SCALE = float(C) ** -0.5

SCALE_ADJ = SCALE
LOG2E = 1.4426950408889634
EXP_K1 = SCALE_ADJ * LOG2E * 128.0
EXP_K2 = 16256.0 - 7.2192745       # 127<<7 + c (c: zero-mean rel err)

F32 = mybir.dt.float32
BF16 = mybir.dt.bfloat16
I16 = mybir.dt.int16
U16 = mybir.dt.uint16


def _build(ctx_dtype=BF16):
    nc = bacc.Bacc("TRN2", target_bir_lowering=False, debug=False, num_devices=B)
    x_d = nc.dram_tensor("x", [N, C], F32, kind="ExternalInput").ap()
    w_d = nc.dram_tensor("W", [OUT, C], F32, kind="ExternalInput").ap()
    b_d = nc.dram_tensor("b", [OUT], F32, kind="ExternalInput").ap()
    o_d = nc.dram_tensor("out", [N, OUT], F32, kind="ExternalOutput").ap()

    with tile.TileContext(nc) as tc:
        with tc.tile_pool(name="const", bufs=1) as const, \
             tc.tile_pool(name="bwork", bufs=6) as bwork, \
             tc.tile_pool(name="ywork", bufs=2) as ywork, \
             tc.tile_pool(name="ps_work", bufs=3, space="PSUM") as ps_work, \
             tc.tile_pool(name="ps_acc", bufs=2, space="PSUM") as ps_acc:

            # running estimate of each exp engine's busy-ns, used to split
            # slabs; DVE also carries z-copies and the per-quad epilogue
            bal = {"s": 0.0, "d": 0.0}

            # ---- setup ----
            # x loads: few big DMAs (each InstDMACopy splits across all 16
            # SDMA slots; many small DMAs pay ~600ns serial issue each)
            x_nat = const.tile([128, NT, 128], F32)       # x tiles [j within tile, c]
            x_view = x_d.rearrange("(t p) c -> p t c", p=128)
            # few DMA chunks (each dma_start pays ~600ns serial issue), small
            # leading ones so the cast/transpose pipeline starts early; the
            # cast/transpose/copy pipeline below is demand-driven per 4-tile
            # group, so the DVE never head-of-line-blocks an early xT copy
            # behind a late cast
            bounds = [0, 4, 12, 22, NT]
            for lo, hi in zip(bounds, bounds[1:]):
                nc.sync.dma_start(x_nat[:, lo:hi, :], x_view[:, lo:hi, :])

            w_sb = const.tile([128, 128], F32)            # W [o, c]
            nc.sync.dma_start(w_sb, w_d)
            bias_bc = const.tile([128, 128], F32)         # bias broadcast to all partitions
            nc.sync.dma_start(bias_bc, bass.AP(tensor=b_d.tensor, offset=b_d.offset,
                                               ap=[[0, 128]] + b_d.ap))

            x_bf = const.tile([128, NT, 128], BF16)
            # all casts upfront: the scheduler runs each as soon as its DMA
            # chunk lands (ready-order beats priority), so they never queue
            # behind main-loop exp work on the DVE
            for g in range(NT // 4):
                nc.vector.tensor_copy(x_bf[:, g * 4:(g + 1) * 4, :],
                                      x_nat[:, g * 4:(g + 1) * 4, :])
                bal["d"] += (58 + 256) / 0.96

            ident = const.tile([128, 128], BF16)
            make_identity(nc, ident)
            xT = const.tile([128, N], BF16)               # [c, n]

            def emit_xT_group(g):
                t_ps = ps_work.tile([128, 512], BF16, name="t_ps", tag="pswork")
                for u in range(4):
                    nc.tensor.transpose(t_ps[:, u * 128:(u + 1) * 128],
                                        x_bf[:, g * 4 + u, :], ident)
                nc.vector.tensor_copy(xT[:, g * 512:(g + 1) * 512], t_ps)
                bal["d"] += (120 + 256) / 0.96

            xT_state = {"emitted": 0}

            def ensure_xT(j_hi):
                need = min(NT // 4, max(1, (j_hi + 3) // 4))
                while xT_state["emitted"] < need:
                    emit_xT_group(xT_state["emitted"])
                    xT_state["emitted"] += 1

            w_bf = const.tile([128, 128], BF16)
            nc.vector.tensor_copy(w_bf, w_sb)
            ensure_xT(4)  # group 0: quad 0's rhs columns
            wt_ps = ps_work.tile([128, 512], BF16, name="t_ps", tag="pswork")
            nc.tensor.transpose(wt_ps[:, 0:128], w_bf, ident)
            wT = const.tile([128, 128], BF16)             # wT[c, o] = W[o, c]
            nc.vector.tensor_copy(wT, wt_ps[:, 0:128])

            # z~ = [x @ W.T | 1]  (bf16), tiled [j within tile, 129]
            zt = const.tile([128, NT, 129], ctx_dtype)
            nc.vector.memset(zt[:, :, 128], 1.0)

            def emit_z_group(g):
                z_ps = ps_work.tile([128, 512], F32, name="z_ps", tag="pswork")
                for u in range(4):
                    j = g * 4 + u
                    nc.tensor.matmul(z_ps[:, u * 128:(u + 1) * 128],
                                     xT[:, j * 128:(j + 1) * 128], wT,
                                     start=True, stop=True)
                nc.vector.tensor_copy(
                    zt[:, g * 4:(g + 1) * 4, 0:128],
                    z_ps.rearrange("p (j c) -> p j c", c=128))
                bal["d"] += (120 + 512) / 0.96

            z_state = {"emitted": 0}

            def ensure_z(j_hi):
                need = min(NT // 4, (j_hi + 3) // 4)
                while z_state["emitted"] < need:
                    emit_z_group(z_state["emitted"])
                    z_state["emitted"] += 1

            zeros128 = const.tile([128, 128], ctx_dtype)
            nc.vector.memset(zeros128, 0.0)
            dummy258 = const.tile([128, 258], ctx_dtype)
            nc.vector.memset(dummy258, 0.0)


            # prefetch a couple of xT/z groups so quad 0's pipeline starts deep
            ensure_xT(8)
            ensure_z(4)

            # ---- main loop ----
            # exp tiles span up to 3 PSUM banks (j-block groups of 3) to
            # amortize the ~352-cycle ACTIVATE overhead.  The four acc
            # accumulators pack two-per-bank: a zero matmul opens the bank's
            # accumulation group (start=True clears has_written bank-wide),
            # then every AV matmul accumulates with start=False.
            # S-matmuls are emitted one group AHEAD so they sit in front of
            # the previous group's AV matmuls in the PE FIFO — otherwise the
            # scalar engine stalls ~1us at every quad boundary (head-of-line
            # blocking behind AVs that wait on exp).
            JG = [2] * 16             # j-block group sizes per quad (sum=32);
                                      # 2-bank slabs, 3 in flight (bufs=3),
                                      # so exp latency jitter never stalls PE
            NB = QW // 128            # i-blocks per quad (4)
            groups = []
            for q in range(NQ):
                jb = 0
                for hi, gsz in enumerate(JG):
                    groups.append((q, jb, gsz, hi))
                    jb += gsz

            s_tiles = {}

            def emit_S(idx):
                q, jb, gsz, hi = groups[idx]
                ensure_xT(jb + gsz)
                s_ps = ps_work.tile([128, QW * gsz], F32, name="s_ps",
                                    tag="pswork")
                for u in range(gsz):
                    j = jb + u
                    nc.tensor.matmul(s_ps[:, u * QW:(u + 1) * QW],
                                     xT[:, j * 128:(j + 1) * 128],
                                     xT[:, q * QW:(q + 1) * QW],
                                     start=True, stop=True)
                s_tiles[idx] = s_ps

            def diag_span(q, jb, gsz):
                lo = hi = None
                for u in range(gsz):
                    j = jb + u
                    if 4 * q <= j < 4 * q + 4:
                        off = u * QW + (j - 4 * q) * 128
                        lo = off if lo is None else lo
                        hi = off + 128
                return lo, hi

            def emit_exp(s_ps, b_sb, q, jb, gsz):
                """Split exp over ScalarE (one range [a,b), must cover the
                diagonal blocks — ~92% of the softmax mass rides on exact
                exp there) and the VectorE bit-trick (the remainder, up to
                two ranges), minimizing the running max busy-ns."""
                FD = gsz * QW
                dlo, dhi = diag_span(q, jb, gsz)
                b_i16 = b_sb.bitcast(I16)
                best = None
                for a in range(0, FD + 1, 128):
                    for b2 in range(a, FD + 1, 128):
                        if dlo is not None and not (a <= dlo and b2 >= dhi):
                            continue
                        ts = bal["s"] + ((172 + (b2 - a)) / 1.2 if b2 > a else 0.0)
                        td = bal["d"] + sum((120 + (h - l)) / 0.96
                                            for l, h in ((0, a), (b2, FD)) if h > l)
                        m = max(ts, td)
                        if best is None or m < best[0]:
                            best = (m, a, b2)
                _, a, b2 = best
                if b2 > a:
                    nc.scalar.activation(b_sb[:, a:b2], s_ps[:, a:b2],
                                         mybir.ActivationFunctionType.Exp,
                                         scale=SCALE_ADJ)
                    bal["s"] += (172 + (b2 - a)) / 1.2
                for l, h in ((0, a), (b2, FD)):
                    if h > l:
                        nc.vector.tensor_scalar(b_i16[:, l:h], s_ps[:, l:h],
                                                EXP_K1, EXP_K2,
                                                op0=mybir.AluOpType.mult,
                                                op1=mybir.AluOpType.add)
                        bal["d"] += (120 + (h - l)) / 0.96

            emit_S(0)
            emit_S(1)
            acc = None
            acc_slice = None
            for idx, (q, jb, gsz, hi) in enumerate(groups):
                if hi == 0:
                    acc = [ps_acc.tile([128, 258], F32, name=f"acc{p}",
                                       tag="acc")
                           for p in range(NB // 2)]

                    def acc_slice(k, w=129, _acc=acc):
                        return _acc[k // 2][:, (k % 2) * 129:(k % 2) * 129 + w]

                if idx + 2 < len(groups):
                    emit_S(idx + 2)
                s_ps = s_tiles.pop(idx)
                b_sb = bwork.tile([128, QW * gsz], ctx_dtype, name="b_sb",
                                  tag="b_sb")
                emit_exp(s_ps, b_sb, q, jb, gsz)
                ensure_z(jb + gsz)
                if hi == 0:
                    for p in range(NB // 2):
                        nc.tensor.matmul(acc[p], zeros128, dummy258,
                                         start=True, stop=False,
                                         skip_group_check=True)
                for u in range(gsz):
                    j = jb + u
                    for k in range(NB):
                        nc.tensor.matmul(
                            acc_slice(k),
                            b_sb[:, u * QW + k * 128:u * QW + (k + 1) * 128],
                            zt[:, j, :], start=False, stop=(j == NT - 1),
                            skip_group_check=True)
                if hi != len(JG) - 1:
                    continue
                # epilogue: y = acc[:, :128] / acc[:, 128] + bias; one DMA/quad
                y4 = ywork.tile([128, NB, 128], F32, name="y4", tag="y4")
                for k in range(NB):
                    rinv = ywork.tile([128, 1], F32, name="rinv", tag="rinv")
                    nc.vector.reciprocal(rinv, acc_slice(k, 129)[:, 128:129])
                    nc.vector.scalar_tensor_tensor(
                        y4[:, k, :], acc_slice(k, 128), rinv, bias_bc,
                        op0=mybir.AluOpType.mult, op1=mybir.AluOpType.add)
                bal["d"] += 2040.0
                o_view = o_d.rearrange("(t p) c -> p t c", p=128)
                nc.sync.dma_start(o_view[:, q * NB:(q + 1) * NB, :], y4)

    nc.compile()
    return nc


_NC_CACHE = {}


def _get_nc():
    if "nc" not in _NC_CACHE:
        _NC_CACHE["nc"] = _build()
    return _NC_CACHE["nc"]


def kernel(x, W, b, _trace=False):
    """x: [8, 4096, 128] f32, W: [128, 128] f32, b: [128] f32 -> [8, 4096, 128] f32."""
    nc = _get_nc()
    x = np.ascontiguousarray(np.asarray(x, dtype=np.float32))
    W = np.ascontiguousarray(np.asarray(W, dtype=np.float32))
    b = np.ascontiguousarray(np.asarray(b, dtype=np.float32))
    in_maps = [{"x": x[i], "W": W, "b": b} for i in range(B)]
    res = bass_utils.run_bass_kernel_spmd(nc, in_maps, core_ids=list(range(B)),
                                          trace=_trace)
    out = np.stack([r["out"] for r in res.results]).astype(np.float32)
    if _trace:
        return out, res
    return out
